# revision 80
# baseline (speedup 1.0000x reference)
"""GQA attention (B=2, T=2048, D=2048, 32 heads / 8 KV groups, head_dim=64,
RoPE, causal) distributed over 8 TRN2 NeuronCores.

Sharding: core i handles batch b = i//4 and KV-group pair (2*(i%4), 2*(i%4)+1),
i.e. 8 query heads + 2 KV heads. QKV is column-sharded, out-proj row-sharded;
each core writes a partial [T, D] output (bf16) and the host sums 4 partials
per batch. No collectives.

v3 design notes (vs the 355us v2; ~308us measured):
 - scores are two CONCURRENT row-tiled K=64 matmuls (h0 on PE rows 0-63, h1 on
   rows 64-127, kp/q stored head-stacked) -> half the score cycles, no q padding
 - softmax exp (ACT, ~1.1us/tile) is hidden by a global work queue: qkv/proj
   matmul quanta are pulled between score and PV emissions at a per-window
   pace (capped at 1/tile so surplus carries forward), and PV trails its
   score by a 3-deep pipeline carried across head-pair boundaries
 - qkv(0) runs ci-major across all 6 output groups at once (2 extra psum
   accumulators borrowed from the idle sc pool), so each arriving DMA eighth
   unlocks 12 matmuls and the DMA-paced ramp keeps the HAM clock warm;
   8 warmup matmuls + a dummy exp (ACT table preload) cover the first ~4us
 - scalar engine is exp-ONLY while attention runs (a single dependent copy on
   the ACT FIFO head stalls every later exp); prologue/epilogue copies may use
   it freely
 - per-chunk reciprocals are split into 4 quarter-token DVE jobs dripped one
   per 3 attention tiles (a monolithic 3.3us reciprocal head-of-line-blocks
   rope copies on the DVE); the last pair's denominators live in their own
   partition-0 tile so the tail chain is short, covered by 12 reserved proj
   quanta
 - wq/xt/wp are host-packed to [128, k, n] and load in a handful of large
   DMAs (205 -> ~70 descriptors); out tiles are [128, 2048] single-DMA stores
   except the epilogue, which stores per-oc to drain early
"""

import sys

sys.path.insert(0, "/opt/trn_rl_repo")

from collections import deque
from contextlib import ExitStack

import numpy as np
import ml_dtypes

from concourse import bacc, mybir, tile
from concourse.bass_utils import run_bass_kernel_spmd

# problem constants (hardcoded per contract)
B, T, D = 2, 2048, 2048
N_HEAD, N_GROUPS, HEAD_DIM = 32, 8, 64
KV_DIM = N_GROUPS * HEAD_DIM  # 512
NCORES = 8
WCOLS = 768  # 512 q + 128 k + 128 v per core

F32 = mybir.dt.float32
BF16 = mybir.dt.bfloat16
TQ = 512  # token chunk
NT = T // TQ  # 4
NCT = D // 128  # 16 contraction tiles for QKV
SCALE = float(HEAD_DIM) ** -0.5


# ---------------------------------------------------------------- host tables


def _host_tables():
    theta = 1.0 / (10000.0 ** (np.arange(0, HEAD_DIM, 2, dtype=np.float64) / HEAD_DIM))
    freqs = np.arange(T, dtype=np.float64)[None, :] * theta[:, None]  # [32, T]
    cos64 = np.repeat(np.cos(freqs), 2, axis=0)  # rows 2i,2i+1 -> cos_i
    sin64 = np.repeat(np.sin(freqs), 2, axis=0)
    sgn = np.where(np.arange(HEAD_DIM) % 2 == 0, -1.0, 1.0)[:, None]
    cos128 = np.concatenate([cos64, cos64], 0)  # [128, T]
    sin128 = np.concatenate([sin64 * sgn, sin64 * sgn], 0)

    swp = np.zeros((128, 128), np.float32)  # swap(q)[d] = q[d^1]
    for d in range(128):
        swp[d ^ 1, d] = 1.0

    kt = np.arange(128)[:, None]
    qt = np.arange(128)[None, :]
    umask = (qt >= kt).astype(np.float32)  # [kt, qt] causal keep-mask
    umask2 = np.stack([umask, umask], axis=1)  # [128, 2, 128] (both heads)

    selb = np.zeros((128, 128), np.float32)  # va_g[kt,d] = v_sb[64g+d, kt]
    for d in range(64):
        selb[d, d] = 1.0  # cols 0-63: group 0
        selb[64 + d, 64 + d] = 1.0  # cols 64-127: group 1

    # sel4[jj]: bcast rows 2jj / 2jj+1 -> psum rows 0-63 / 64-127
    sel4 = np.zeros((4, 128, 128), np.float32)
    for jj in range(4):
        sel4[jj, 2 * jj, :64] = 1.0
        sel4[jj, 2 * jj + 1, 64:] = 1.0
    bf = ml_dtypes.bfloat16
    return (cos128.astype(bf), sin128.astype(bf), swp.astype(bf), umask2.astype(bf),
            selb.astype(bf), sel4.astype(bf))


def _shard_inputs(x, w_qkv, w_proj):
    """Per-core input dicts. Core i: batch i//4, group pair gp = i%4."""
    cos128, sin128, swp, umask2, selb, sel4 = _host_tables()
    bf = ml_dtypes.bfloat16
    # xt packed [128, 16, T]: xtp[p, ci, t] = x[b].T[128*ci+p, t]
    xtp = []
    for b in range(B):
        xt = x[b].T.astype(bf)  # [D, T]
        xtp.append(np.ascontiguousarray(xt.reshape(NCT, 128, T).transpose(1, 0, 2)))
    maps = []
    for i in range(NCORES):
        b, gp = i // 4, i % 4
        heads = [8 * gp + j for j in range(8)]  # global heads of this core
        # q blocks pair local heads (j, j+4) = (group 2gp head j, group 2gp+1 head j)
        qcols = []
        for j in range(4):
            qcols.append(w_qkv[:, 64 * heads[j] : 64 * heads[j] + 64])
            qcols.append(w_qkv[:, 64 * heads[j + 4] : 64 * heads[j + 4] + 64])
        kcol = w_qkv[:, D + 128 * gp : D + 128 * gp + 128]
        vcol = w_qkv[:, D + KV_DIM + 128 * gp : D + KV_DIM + 128 * gp + 128]
        wq = np.concatenate(qcols + [kcol, vcol], axis=1).astype(bf)  # [D, 768]
        wqp = np.ascontiguousarray(wq.reshape(NCT, 128, WCOLS).transpose(1, 0, 2))
        # w_proj rows in ypair order: pair j = [head j ; head j+4]
        wrows = []
        for j in range(4):
            wrows.append(w_proj[64 * heads[j] : 64 * heads[j] + 64, :])
            wrows.append(w_proj[64 * heads[j + 4] : 64 * heads[j + 4] + 64, :])
        wp = np.concatenate(wrows, axis=0).astype(bf)  # [512, D]
        wpp = np.ascontiguousarray(wp.reshape(4, 128, D).transpose(1, 0, 2))
        maps.append(
            {
                "xt": xtp[b],
                "wqkv": wqp,
                "wproj": wpp,
                "costab": cos128,
                "sintab": sin128,
                "swp": swp,
                "umask2": umask2,
                "selb": selb,
                "sel4": sel4,
            }
        )
    return maps


# ------------------------------------------------------------------- builder


def build_nc():
    nc = bacc.Bacc("TRN2", target_bir_lowering=False, debug=False, num_devices=NCORES)
    xt_d = nc.dram_tensor("xt", [128, NCT, T], BF16, kind="ExternalInput").ap()
    wq_d = nc.dram_tensor("wqkv", [128, NCT, WCOLS], BF16, kind="ExternalInput").ap()
    wp_d = nc.dram_tensor("wproj", [128, 4, D], BF16, kind="ExternalInput").ap()
    cos_d = nc.dram_tensor("costab", [128, T], BF16, kind="ExternalInput").ap()
    sin_d = nc.dram_tensor("sintab", [128, T], BF16, kind="ExternalInput").ap()
    swp_d = nc.dram_tensor("swp", [128, 128], BF16, kind="ExternalInput").ap()
    um2_d = nc.dram_tensor("umask2", [128, 2, 128], BF16, kind="ExternalInput").ap()
    slb_d = nc.dram_tensor("selb", [128, 128], BF16, kind="ExternalInput").ap()
    s4_d = nc.dram_tensor("sel4", [4, 128, 128], BF16, kind="ExternalInput").ap()
    out_d = nc.dram_tensor("out", [T, D], BF16, kind="ExternalOutput").ap()

    with (
        nc.allow_low_precision(reason="bf16 matmul operands; fp32 psum accumulation"),
        tile.TileContext(nc) as tc,
        ExitStack() as ctx,
    ):
        const = ctx.enter_context(tc.tile_pool(name="const", bufs=1))
        keep = ctx.enter_context(tc.tile_pool(name="keep", bufs=1))
        p_x = ctx.enter_context(tc.tile_pool(name="p_x", bufs=2))
        p_w = ctx.enter_context(tc.tile_pool(name="p_w", bufs=3))
        p_yh = ctx.enter_context(tc.tile_pool(name="p_yh", bufs=3))
        p_pt = ctx.enter_context(tc.tile_pool(name="p_pt", bufs=5))
        ps_a = ctx.enter_context(tc.tile_pool(name="ps_a", bufs=2, space="PSUM"))
        ps_sc = ctx.enter_context(tc.tile_pool(name="ps_sc", bufs=2, space="PSUM"))
        ps_pv = ctx.enter_context(tc.tile_pool(name="ps_pv", bufs=1, space="PSUM"))

        warm_t = const.tile([128, TQ], BF16)
        cos_t = const.tile([128, T], BF16)
        sin_t = const.tile([128, T], BF16)
        swp_t = const.tile([128, 128], BF16)
        um2_t = const.tile([128, 2, 128], BF16)
        slb_t = const.tile([128, 128], BF16)
        s4_t = const.tile([128, 4, 128], BF16)
        wq_sb = keep.tile([128, NCT, WCOLS], BF16, tag="wq", name="wq_sb")
        wp_sb = keep.tile([128, 4, D], BF16, tag="wp", name="wp_sb")

        # persistent per-chunk activations; q pairs head-stacked [h0; h1]
        qp_c = [
            [keep.tile([128, TQ], BF16, tag=f"qp{c}_{jp}", name=f"qp{c}_{jp}") for jp in range(4)]
            for c in range(NT)
        ]
        kp_c = [keep.tile([128, TQ], BF16, tag=f"kp{c}", name=f"kp{c}") for c in range(NT)]
        v_c = [keep.tile([128, TQ], BF16, tag=f"v{c}", name=f"v{c}") for c in range(NT)]
        va_c = [
            [keep.tile([128, 4, 65], BF16, tag=f"va{c}_{g}", name=f"va{c}_{g}") for g in range(2)]
            for c in range(NT)
        ]
        yp_c = [
            [keep.tile([128, TQ], BF16, tag=f"yp{c}_{jj}", name=f"yp{c}_{jj}") for jj in range(4)]
            for c in range(NT)
        ]
        sump_c = [keep.tile([128, TQ], BF16, tag=f"sump{c}", name=f"sump{c}") for c in range(NT)]
        # chunk 3 pair 3's denominators get their own tile (rows 0:1) so the
        # tail reciprocal / broadcast stay partition-0-aligned
        sump3b = keep.tile([128, TQ], BF16, tag="sump3b", name="sump3b")

        # sump rows 8-127 are never written but are read by the bcast matmul
        # (times zero stationary cols); they must be finite. va cols 64 are the
        # static ones-columns (denominator trick); warm_t feeds warmup matmuls.
        nc.gpsimd.memset(warm_t[:], 0.0)
        nc.gpsimd.memset(sump3b[:], 1.0)
        for c in range(NT):
            nc.gpsimd.memset(sump_c[c][:], 1.0)
            for g in range(2):
                nc.gpsimd.memset(va_c[c][g][:, :, 64:65], 1.0)

        xts = [None] * NT  # xt sbuf tiles [128, NCT, TQ] per chunk

        # ---------------- work queue: qkv (hi) / proj (lo) matmul quanta
        q_hi = deque()
        q_lo = deque()
        epi = [False]  # True once all exps are emitted (epilogue)

        def pull(n, lo_ok=True):
            for _ in range(n):
                if q_hi:
                    q_hi.popleft()()
                elif lo_ok and q_lo:
                    q_lo.popleft()()
                else:
                    break

        def drain_hi():
            while q_hi:
                q_hi.popleft()()

        # ---------------- DMA helpers (host-packed layouts, quarter DMAs so
        # qkv quanta unblock progressively as data arrives)
        def dma_xt(c):
            xt_t = p_x.tile([128, NCT, TQ], BF16, tag="xt", name=f"xt{c}")
            for g in range(4):
                nc.sync.dma_start(
                    xt_t[:, 4 * g : 4 * g + 4, :],
                    xt_d[:, 4 * g : 4 * g + 4, TQ * c : TQ * c + TQ],
                )
            xts[c] = xt_t

        # ---------------- qkv + rope
        def rope_rest(c, raw, dest):
            ts = slice(TQ * c, TQ * c + TQ)
            sw = ps_a.tile([128, TQ], F32, tag="acc", name="sw")
            nc.tensor.matmul(sw[:], swp_t[:], raw[:], start=True, stop=True)
            t1 = p_w.tile([128, TQ], BF16, tag="t1", name="t1")
            t2 = p_w.tile([128, TQ], BF16, tag="t2", name="t2")
            nc.vector.tensor_mul(t1[:], raw[:], cos_t[:, ts])
            nc.vector.tensor_mul(t2[:], sw[:], sin_t[:, ts])
            nc.vector.tensor_add(dest[:], t1[:], t2[:])

        def qkv_mms(c, oc, lo, hi, ps):
            for ci in range(lo, hi):
                nc.tensor.matmul(
                    ps[:],
                    wq_sb[:, ci, 128 * oc : 128 * oc + 128],
                    xts[c][:, ci, :],
                    start=(ci == 0),
                    stop=(ci == NCT - 1),
                )

        def qkv_post(c, oc, hold):
            if oc < 5:
                rope_rest(c, hold["raw"], qp_c[c][oc] if oc < 4 else kp_c[c])
            else:
                for kloc in range(4):
                    vp = ps_a.tile([128, 128], F32, tag="acc", name="vp")
                    nc.tensor.matmul(
                        vp[:],
                        v_c[c][:, 128 * kloc : 128 * kloc + 128],
                        slb_t[:],
                        start=True,
                        stop=True,
                    )
                    nc.vector.tensor_copy(va_c[c][0][:, kloc, 0:64], vp[:, 0:64])
                    nc.vector.tensor_copy(va_c[c][1][:, kloc, 0:64], vp[:, 64:128])

        def enqueue_qkv(c, ocs=(4, 5, 0, 1, 2, 3)):
            # per oc: 4 quanta of 4 accumulation mms (the last one also starts
            # the PSUM evacuation), then a separate quantum for the rope / v
            # post-processing so its PE ops (swap matmul, vp) land a quantum
            # later than the DVE copy they wait on. The psum tile is created
            # inside the FIRST quantum so ps_a slot-rotation order matches
            # emission order (no WAR deadlocks).
            # each oc's post quantum is enqueued AFTER the next oc's matmul
            # quanta, so its PSUM-evac copy has ~4 tiles of DVE lead time
            # before the swap matmul needs it
            prev_post = None
            for oc in ocs:
                hold = {}
                for lo in range(0, NCT, 4):
                    def quant(c=c, oc=oc, lo=lo, hold=hold):
                        if lo == 0:
                            hold["ps"] = ps_a.tile(
                                [128, TQ], F32, tag="acc", name=f"qkv{c}_{oc}"
                            )
                        qkv_mms(c, oc, lo, lo + 4, hold["ps"])
                        if lo + 4 == NCT:
                            if oc == 5:
                                nc.vector.tensor_copy(v_c[c][:], hold["ps"][:])
                            else:
                                raw = p_w.tile([128, TQ], BF16, tag="raw", name="raw")
                                nc.vector.tensor_copy(raw[:], hold["ps"][:])
                                hold["raw"] = raw
                    q_hi.append(quant)
                if prev_post is not None:
                    q_hi.append(prev_post)
                prev_post = lambda c=c, oc=oc, hold=hold: qkv_post(c, oc, hold)
            q_hi.append(prev_post)

        # ---------------- attention
        def emit_sc(s, jp, kj):
            qcs, kloc = kj // 4, kj % 4
            col0 = max(kj * 128 - s * TQ, 0)
            sc = ps_sc.tile([128, 2, TQ], F32, tag="sc", name="sc")
            for h in range(2):
                nc.tensor.matmul(
                    sc[:, h, col0:TQ],
                    kp_c[qcs][64 * h : 64 * h + 64, 128 * kloc : 128 * kloc + 128],
                    qp_c[s][jp][64 * h : 64 * h + 64, col0:TQ],
                    start=True,
                    stop=True,
                )
            pt = p_pt.tile([128, 2, TQ], BF16, tag="pt", name="pt")
            nc.scalar.activation(
                pt[:, :, col0:TQ],
                sc[:, :, col0:TQ],
                mybir.ActivationFunctionType.Exp,
                scale=SCALE,
            )
            if kj >= 4 * s:  # diagonal tile: triangular keep-mask, both heads
                nc.gpsimd.tensor_mul(
                    pt[:, :, col0 : col0 + 128],
                    pt[:, :, col0 : col0 + 128],
                    um2_t[:],
                )
            return pt, col0

        def emit_pv(s, jp, kj, pv, pt, col0, last):
            qcs, kloc = kj // 4, kj % 4
            for h in range(2):
                nc.tensor.matmul(
                    pv[0:65, h, col0:TQ],
                    va_c[qcs][h][:, kloc, :],
                    pt[:, h, col0:TQ],
                    start=(kj == 0),
                    stop=last,
                )

        def attn_evac(s, jp, pv):
            # y of h0 -> yp rows 0-63 directly; h1 via staging + partition-move
            # DMA; both denominators in one copy + one DMA into sump rows.
            # The very last pair's copies ride the scalar engine (no exp left)
            # to keep the epilogue's DVE free for the norm chain.
            last = s == NT - 1 and jp == 3
            cp = nc.scalar.copy if last else nc.vector.tensor_copy
            cp(yp_c[s][jp][0:64, :], pv[0:64, 0, :])
            yh = p_yh.tile([64, TQ], BF16, tag="yh", name="yh")
            cp(yh[0:64, :], pv[0:64, 1, :])
            nc.sync.dma_start(yp_c[s][jp][64:128, :], yh[0:64, :])
            dn = p_yh.tile([65, 2, TQ], BF16, tag="dn", name="dn")
            cp(dn[64:65, :, :], pv[64:65, :, :])
            dst = sump3b[0:2, :] if last else sump_c[s][2 * jp : 2 * jp + 2, :]
            nc.sync.dma_start(dst, dn[64:65, :, :])

        cr = [0.0]  # fractional filler-pull credit, persists across windows

        def attn_window(s, pace, lo_ok, hooks=None, chores=None):
            # 2-deep sc->exp->pv software pipeline, carried across head-pair
            # boundaries: pv(kj) trails sc(kj) by two tiles, so the ~1us exp
            # latency never blocks the PE even with no filler available
            # (ACT-paced windows), and there is no per-pair pipe refill.
            nkj = 4 * s + 4

            def paced_pull():
                cr[0] += pace
                n = int(cr[0])
                cr[0] -= n
                pull(n, lo_ok)

            pend = []
            pvt = {}
            tcnt = [0]
            done = {jp: 0 for jp in range(4)}

            def pop_one():
                jp, kj, pt, col0 = pend.pop(0)
                done[jp] += 1
                emit_pv(s, jp, kj, pvt[jp], pt, col0, done[jp] == nkj)
                if done[jp] == nkj:
                    attn_evac(s, jp, pvt[jp])
                    if hooks and jp in hooks:
                        hooks[jp]()

            # kj=0 first (full-range accumulation start), then the small
            # diagonal tiles (their exps amortize the ACT overhead poorly --
            # interleave them with full tiles instead of clustering at the
            # pair's end), then the remaining full tiles
            korder = [0] + list(range(4 * s, nkj)) + list(range(1, 4 * s))
            if s == 0:
                korder = list(range(nkj))
            for jp in range(4):
                pvt[jp] = ps_pv.tile([128, 2, TQ], F32, tag="pv", name="pv")
                for kj in korder:
                    if chores and tcnt[0] % 3 == 0:
                        chores.pop(0)()  # small DVE jobs, spaced 3 tiles apart
                    tcnt[0] += 1
                    if len(pend) >= 3:
                        paced_pull()
                        pop_one()
                    pend.append((jp, kj, *emit_sc(s, jp, kj)))
            while pend:
                paced_pull()
                pop_one()

        # ---------------- normalization + projection
        def recip_chores(c):
            # the 3.3us reciprocal would head-of-line-block the DVE right at a
            # window boundary; split it into four quarter-token jobs instead
            return [
                (lambda c=c, q=q: nc.vector.reciprocal(
                    sump_c[c][0:8, 128 * q : 128 * q + 128],
                    sump_c[c][0:8, 128 * q : 128 * q + 128],
                ))
                for q in range(4)
            ]

        def norm_bcast(c, jjs=(0, 1, 2, 3)):
            for jj in jjs:
                bc = ps_a.tile([128, TQ], F32, tag="acc", name="bc")
                nc.tensor.matmul(
                    bc[:], s4_t[:, jj, :], sump_c[c][:], start=True, stop=True
                )
                nc.vector.tensor_mul(yp_c[c][jj][:], yp_c[c][jj][:], bc[:])

        def proj_unit(c, tb, oc, hold):
            if oc == 0:
                hold["ot"] = p_w.tile(
                    [128, D], BF16, tag="ot", name=f"ot{c}_{tb}", bufs=3
                )
                hold["epi"] = epi[0]  # store mode fixed per tb
            ot = hold["ot"]
            pj = ps_a.tile([128, TQ], F32, tag="acc", name="pj")
            for jj in range(4):
                nc.tensor.matmul(
                    pj[:],
                    yp_c[c][jj][:, 128 * tb : 128 * tb + 128],
                    wp_sb[:, jj, TQ * oc : TQ * oc + TQ],
                    start=(jj == 0),
                    stop=(jj == 3),
                )
            tok0 = c * TQ + tb * 128
            if hold["epi"]:
                # epilogue (all exps done): alternate evac engines AND DMA
                # issue queues, store per oc so the out stores drain while the
                # remaining proj mms run (16 serial issues on one queue would
                # add ~9us of drain tail)
                if oc % 2 == 1:
                    nc.scalar.copy(ot[:, TQ * oc : TQ * oc + TQ], pj[:])
                else:
                    nc.vector.tensor_copy(ot[:, TQ * oc : TQ * oc + TQ], pj[:])
                nc.sync.dma_start(
                    out_d[tok0 : tok0 + 128, TQ * oc : TQ * oc + TQ],
                    ot[:, TQ * oc : TQ * oc + TQ],
                )
            else:
                nc.vector.tensor_copy(ot[:, TQ * oc : TQ * oc + TQ], pj[:])
                if oc == 3:
                    nc.sync.dma_start(out_d[tok0 : tok0 + 128, :], ot[:])

        def enqueue_proj(c):
            for tb in range(4):
                hold = {}
                for oc in range(4):
                    q_lo.append(
                        lambda c=c, tb=tb, oc=oc, hold=hold: proj_unit(c, tb, oc, hold)
                    )

        # ---------------- emission
        # warmup matmuls: un-throttle the HAM clock during the initial DMA
        # wait; a dummy exp pulls the ~2.7us ACT table load into the ramp too
        wps = ps_a.tile([128, TQ], F32, tag="acc", name="warm")
        for wi in range(8):
            nc.tensor.matmul(
                wps[:], warm_t[:, 0:128], warm_t[:], start=(wi == 0), stop=(wi == 7)
            )
        wexp = p_w.tile([128, 8], BF16, tag="t1", name="wexp")
        nc.scalar.activation(
            wexp[:], warm_t[:, 0:8], mybir.ActivationFunctionType.Exp, scale=SCALE
        )

        # critical DMAs first: wq + xt(0) interleaved in eighths, so each PE
        # wait during the DMA-bound ramp stays well under the ~3.4us HAM
        # re-throttle window
        for g in range(8):
            nc.sync.dma_start(wq_sb[:, 2 * g : 2 * g + 2, :], wq_d[:, 2 * g : 2 * g + 2, :])
            xt_t = (
                p_x.tile([128, NCT, TQ], BF16, tag="xt", name="xt0") if g == 0 else xts[0]
            )
            xts[0] = xt_t
            nc.sync.dma_start(
                xt_t[:, 2 * g : 2 * g + 2, :], xt_d[:, 2 * g : 2 * g + 2, 0:TQ]
            )
        dma_xt(1)
        nc.sync.dma_start(cos_t[:], cos_d)
        nc.sync.dma_start(sin_t[:], sin_d)
        nc.sync.dma_start(swp_t[:], swp_d)
        nc.sync.dma_start(um2_t[:], um2_d)
        nc.sync.dma_start(slb_t[:], slb_d)
        for jj in range(4):
            nc.sync.dma_start(s4_t[:, jj, :], s4_d[jj, :, :])
        nc.sync.dma_start(wp_sb[:], wp_d)

        # qkv(0) runs ci-major across ALL SIX output groups at once: each
        # arriving (wq, xt) eighth unlocks 12 matmuls (~2.6us of PE work), so
        # the DMA-paced ramp keeps the PE fed and the HAM clock warm. The two
        # extra accumulators borrow the (still idle) ps_sc banks.
        acc45 = [
            ps_a.tile([128, TQ], F32, tag="acc", name=f"qkv0_{oc}") for oc in (4, 5)
        ]
        accq = [
            ps_sc.tile([128, 2, TQ], F32, tag="sc", name=f"qkv0_q{i}") for i in range(2)
        ]
        acc0 = {
            4: acc45[0], 5: acc45[1],
            0: accq[0][:, 0, :], 1: accq[0][:, 1, :],
            2: accq[1][:, 0, :], 3: accq[1][:, 1, :],
        }
        for ci in range(NCT):
            for oc in (4, 5, 0, 1, 2, 3):
                nc.tensor.matmul(
                    acc0[oc][:],
                    wq_sb[:, ci, 128 * oc : 128 * oc + 128],
                    xts[0][:, ci, :],
                    start=(ci == 0),
                    stop=(ci == NCT - 1),
                )
        # prologue evacuations: k and q0 ride the DVE (shortest path to the
        # first attention tile), the rest go to the still-idle scalar engine.
        # qkv(1) quanta are pulled between the posts so the PE has work while
        # each post's DVE chain completes.
        enqueue_qkv(1)
        for oc in (4, 5, 0, 1, 2, 3):
            hold = {"ps": acc0[oc]}
            if oc == 5:
                nc.scalar.copy(v_c[0][:], acc0[oc][:])
            else:
                raw = p_w.tile([128, TQ], BF16, tag="raw", name="raw")
                if oc in (4, 0):
                    nc.vector.tensor_copy(raw[:], acc0[oc][:])
                else:
                    nc.scalar.copy(raw[:], acc0[oc][:])
                hold["raw"] = raw
            qkv_post(0, oc, hold)
            pull(2)

        for s in range(NT):
            if s >= 1:
                drain_hi()  # qkv(s) leftovers must finish before attn(s)
            if s + 2 < NT:
                dma_xt(s + 2)  # after drain: its p_x slot's old readers are emitted
            if s + 1 < NT and s >= 1:
                enqueue_qkv(s + 1)
            ch = recip_chores(s - 1) if s >= 1 else None
            # filler supply this window: qkv(s+1) quanta (+ proj(s-1) units,
            # except in window 2, where proj is deferred to window 3 to match
            # window 3's large exp batch); spread evenly over attention tiles
            lo_ok = True  # w2-block test
            navail = len(q_hi) + len(q_lo)
            if s >= 1 and lo_ok:
                navail += 16  # proj(s-1), enqueued after attn(s, 0)
            if not lo_ok:
                navail = len(q_hi)
            if s == NT - 1:
                navail = max(0, navail - 12)  # reserve PE work for the norm gap
            # cap at 1 filler quantum per tile: surplus carries to the next
            # window (via the deques), smoothing supply across all windows
            pace = min(navail / (4 * (4 * s + 4)), 1.0)

            hooks = {}
            if s >= 1:
                def post_jp0(s=s):
                    norm_bcast(s - 1)
                    enqueue_proj(s - 1)
                hooks[0] = post_jp0
            if s == NT - 1:
                # the last chunk's pairs 0-2 reciprocal runs during pair 3's
                # attention (DVE only -- no PE instruction, no queue block);
                # rows 6:8 of sump hold 1.0 then, pair 3 lives in sump3b
                hooks[2] = lambda: nc.vector.reciprocal(
                    sump_c[NT - 1][0:8, :], sump_c[NT - 1][0:8, :]
                )
            attn_window(s, pace, lo_ok, hooks, chores=ch)
        epi[0] = True
        # norm muls for pairs 0-2 first (their recip ran in the jp2 hook),
        # then the pair-3 reciprocal, with the reserved proj units covering
        # the DVE latency; pair-3's bcast+mul, then the final projection
        norm_bcast(NT - 1, jjs=(0, 1, 2))
        nc.vector.reciprocal(sump3b[0:2, :], sump3b[0:2, :])
        while q_lo:
            q_lo.popleft()()
        bc3 = ps_a.tile([128, TQ], F32, tag="acc", name="bc3")
        nc.tensor.matmul(bc3[:], s4_t[:, 0, :], sump3b[:], start=True, stop=True)
        nc.vector.tensor_mul(yp_c[NT - 1][3][:], yp_c[NT - 1][3][:], bc3[:])
        enqueue_proj(NT - 1)
        while q_lo:
            q_lo.popleft()()

    nc.compile()
    return nc


_NC_CACHE = None


def _get_nc():
    global _NC_CACHE
    if _NC_CACHE is None:
        _NC_CACHE = build_nc()
    return _NC_CACHE


def kernel(x, w_qkv, w_proj, _trace=False, _nc=None):
    x = np.asarray(x, np.float32)
    w_qkv = np.asarray(w_qkv, np.float32)
    w_proj = np.asarray(w_proj, np.float32)
    nc = _nc if _nc is not None else _get_nc()
    in_maps = _shard_inputs(x, w_qkv, w_proj)
    res = run_bass_kernel_spmd(nc, in_maps, core_ids=list(range(NCORES)), trace=_trace)
    out = np.zeros((B, T, D), np.float32)
    for i in range(NCORES):
        out[i // 4] += res.results[i]["out"].astype(np.float32)
    if _trace:
        return out, res
    return out


if __name__ == "__main__":
    rng = np.random.default_rng(0)
    x = rng.standard_normal((B, T, D), dtype=np.float32)
    wq = rng.standard_normal((D, D + 2 * KV_DIM), dtype=np.float32) * D**-0.5
    wp = rng.standard_normal((D, D), dtype=np.float32) * D**-0.5
    y = kernel(x, wq, wp)
    print(y.shape, y.dtype)
